# revision 15
# baseline (speedup 1.0000x reference)
"""ChordProgressionLoss Trainium2 kernel.

Strategy (pure data parallel over the time axis, 8 cores):
  - T = 1,000,000 rows of 4 notes (pred + targ). Each core handles 125,000
    consecutive "positions" (timesteps / window starts).
  - Per core, time is laid out as P=125 SBUF partitions x S=1000 positions,
    each partition holding a contiguous chunk of F=S+3 rows (3-row halo so
    the 4-step progression windows never cross a partition).
  - Per row the kernel computes, with exact small-integer arithmetic:
      pc       = note mod 12 (rounding-agnostic int-convert trick)
      M_c      = presence of pitch-class c (10 classes used by templates)
      s_p, s_t = distinct-class counts via pairwise-equality dedup
      inter    = |Pset cap Tset| via pairwise equality + dedup weights
      I_tau    = |Pset cap template| for the 6 distinct template chords
      q_tau    = I_tau / (s_p + 3 - I_tau)  (= 1 - Jaccard distance)
      sim      = (inter + eps*(s_p+s_t) + 12 eps^2) / ((s_p+12eps)(s_t+12eps))
  - Window terms: min(maj, mino) = 1 - max(QM, Qm)/4 where
      QM[i] = qM1[i] + qM2[i+1] + qM3[i+2] + qM1[i+3]  (same for minor),
    accumulated per partition with on-instruction accumulators.
  - Device returns per-partition sums [125, 2] (sim sum, window-max sum);
    the host reduces across partitions/cores in float64 and fixes up the
    3 windows that fall off the end of the sequence (they are computed from
    zero-padded rows on core 7 and subtracted here).
"""

import sys

sys.path.insert(0, "/opt/trn_rl_repo")

import numpy as np

_NCORES = 8
_T = 1_000_000
_PERCORE = _T // _NCORES  # 125000

# Device geometry (full problem)
_P = 125      # partitions used
_S = 1000     # positions per partition
_F = _S + 3   # rows per partition (halo 3)

_EPS = 1.0e-6

# 6 distinct template chords (major I, IV, V; minor i, iv, v)
_TEMPLATES = [
    (0, 4, 7), (5, 9, 0), (7, 11, 2),   # major
    (0, 3, 7), (5, 8, 0), (7, 10, 2),   # minor
]
_CLASSES = sorted({c for t in _TEMPLATES for c in t})  # 10 classes


def build_nc(P=_P, S=_S):
    """Build the per-core Bass program (SPMD: same program on all cores)."""
    from concourse import bass, mybir
    from concourse.bass import AP

    F32 = mybir.dt.float32
    I32 = mybir.dt.int32
    ALU = mybir.AluOpType

    F = S + 3
    assert S % 2 == 0
    W = S // 2          # window positions per pass
    L = W + 3           # rows needed per pass
    n_in = (P - 1) * S + F  # rows per core

    nc = bass.Bass("TRN2", target_bir_lowering=False, debug=False,
                   detect_race_conditions=False)
    # const bias APs for the ScalarE activations (pattern from Bass.__init__)
    for _val in sorted({-float(c) for c in _CLASSES if c} | {1.0}):
        if (F32, _val) in nc.const_aps.aps:
            continue
        _ct = nc.alloc_sbuf_tensor(f"const-float32-{_val}", [128, 1], F32)
        nc.gpsimd.memset(_ct.ap(), _val)
        nc.const_aps.aps[(F32, _val)] = _ct.ap()
    nc.all_engine_barrier()
    xp_d = nc.declare_dram_parameter("xp", [n_in * 4], F32, isOutput=False)
    xt_d = nc.declare_dram_parameter("xt", [n_in * 4], F32, isOutput=False)
    out_d = nc.declare_dram_parameter("out", [2, P], F32, isOutput=True)

    sb = {}

    def t4(nm, dt=F32):  # [P, 4F] full-chunk tile
        sb[nm] = nc.alloc_sbuf_tensor(nm, [128, 4 * F], dt)
        return sb[nm].ap()

    def tL(nm, dt=F32):  # [P, L] per-pass series tile
        sb[nm] = nc.alloc_sbuf_tensor(nm, [128, L], dt)
        return sb[nm].ap()

    def t1(nm, dt=F32):  # [P, 1] accumulator
        sb[nm] = nc.alloc_sbuf_tensor(nm, [128, 1], dt)
        return sb[nm].ap()

    xp, xt = t4("xp_s"), t4("xt_s")

    def tz(nm, dt=F32):  # [P, 4L] per-pass chunk tile
        sb[nm] = nc.alloc_sbuf_tensor(nm, [128, 4 * L], dt)
        return sb[nm].ap()

    pcp, pct = tz("pcp"), tz("pct")
    ii = tz("ii", I32)
    rr = tz("rr")
    ng = tz("ng")
    z = tz("z")
    z2 = tz("z2")
    zsq = tz("zsq")

    M = {c: tL(f"M{c}") for c in _CLASSES}
    I6 = [tL(f"I{j}") for j in range(6)]
    sc1, sc2, sc3, sc4, sc5 = (tL(f"sc{k}") for k in range(1, 6))
    eA, eB, eC, eD, eE = (tL(f"e{k}") for k in "ABCDE")
    e01t, m3t, m4t = tL("e01t"), tL("m3t"), tL("m4t")
    msumP, msumT = tL("msumP"), tL("msumT")
    Sh1, Sh2, Sw1, Sw2, hk, pw = (tL(nm) for nm in
                                  ["Sh1", "Sh2", "Sw1", "Sw2", "hk", "pw"])
    inter = tL("inter")
    # aliases into buffers that are dead by the time these are used
    rp, rt = hk, pw            # F-stage: E scratch is free
    gw, gu, gg = Sh1, Sh2, Sw1  # G-stage: E accumulators are free
    q6 = [tL(f"q{j}") for j in range(6)]
    QM, Qm = tL("QM"), tL("Qm")
    Qa, Qb, Qc, Qd = eA, eB, eC, eD  # H-stage: compare scratch is free

    sim_acc = [t1("sim_a0"), t1("sim_a1")]
    win_acc = [t1("win_a0"), t1("win_a1")]
    tot = t1("tot_sim"), t1("tot_win")

    with (
        nc.Block() as block,
        nc.semaphore("dma_sem") as dma_sem,
        nc.semaphore("dma2_sem") as dma2_sem,
        nc.semaphore("odma_sem") as odma_sem,
        nc.semaphore("s_pc") as s_pc,
        nc.semaphore("s_z") as s_z,
        nc.semaphore("s_zread") as s_zread,
        nc.semaphore("v_sem") as v_sem,
    ):

        @block.sync
        def _(s):
            c1 = 4 * L                    # columns needed by pass 0
            s.dma_start(out=xp[:P, :c1], in_=AP(xp_d, 0, [[4 * S, P], [1, c1]])).then_inc(dma_sem, 16)
            s.dma_start(out=xt[:P, :c1], in_=AP(xt_d, 0, [[4 * S, P], [1, c1]])).then_inc(dma_sem, 16)
            s.dma_start(out=xp[:P, c1:], in_=AP(xp_d, c1, [[4 * S, P], [1, 4 * F - c1]])).then_inc(dma2_sem, 16)
            s.dma_start(out=xt[:P, c1:], in_=AP(xt_d, c1, [[4 * S, P], [1, 4 * F - c1]])).then_inc(dma2_sem, 16)

        @block.vector
        def _(v):
            v.wait_ge(dma_sem, 32)

            for ipass, fs in enumerate((0, W)):
                if ipass == 1:
                    v.wait_ge(dma2_sem, 32)
                # ---- stage A: pc = note mod 12 (rounding-agnostic) ----
                for srcf, dst in ((xp, pcp), (xt, pct)):
                    sl = srcf[:P, 4 * fs: 4 * (fs + L)]
                    v.tensor_scalar(out=ii[:P], in0=sl, scalar1=float(1.0 / 12.0),
                                    scalar2=0.0, op0=ALU.mult, op1=ALU.add)
                    v.scalar_tensor_tensor(out=rr[:P], in0=ii[:P], scalar=-12.0,
                                           in1=sl, op0=ALU.mult, op1=ALU.add)
                    v.tensor_single_scalar(out=ng[:P], in_=rr[:P], scalar=0.0, op=ALU.is_lt)
                    ins_pc = v.scalar_tensor_tensor(out=dst[:P], in0=ng[:P], scalar=12.0,
                                           in1=rr[:P], op0=ALU.mult, op1=ALU.add)
                    if dst is pcp:
                        ins_pc.then_inc(s_pc, 1)

                pc4p = pcp[:P]       # [P, 4L] contiguous
                pc4t = pct[:P]
                pvp = pc4p.rearrange("p (f i) -> p f i", i=4)
                pvt = pc4t.rearrange("p (f i) -> p f i", i=4)
                Pn = [pvp[:, :, i] for i in range(4)]
                Tn = [pvt[:, :, i] for i in range(4)]

                # ---- D: dedup msum for pred and targ ----
                # pred
                v.tensor_tensor(out=eA[:P], in0=Pn[0], in1=Pn[1], op=ALU.is_equal)  # e01
                v.tensor_tensor(out=eB[:P], in0=Pn[0], in1=Pn[2], op=ALU.is_equal)  # e02
                v.tensor_tensor(out=eC[:P], in0=Pn[1], in1=Pn[2], op=ALU.is_equal)  # e12
                v.tensor_tensor(out=eD[:P], in0=eB[:P], in1=eC[:P], op=ALU.max)     # m3
                v.tensor_tensor(out=eB[:P], in0=Pn[0], in1=Pn[3], op=ALU.is_equal)  # e03
                v.tensor_tensor(out=eC[:P], in0=Pn[1], in1=Pn[3], op=ALU.is_equal)  # e13
                v.tensor_tensor(out=eE[:P], in0=eB[:P], in1=eC[:P], op=ALU.max)     # m4a
                v.tensor_tensor(out=eB[:P], in0=Pn[2], in1=Pn[3], op=ALU.is_equal)  # e23
                v.tensor_tensor(out=eC[:P], in0=eE[:P], in1=eB[:P], op=ALU.max)     # m4
                v.tensor_tensor(out=eE[:P], in0=eA[:P], in1=eD[:P], op=ALU.add)     # e01+m3
                v.tensor_tensor(out=msumP[:P], in0=eE[:P], in1=eC[:P], op=ALU.add)
                # targ (keep e01t, m3t, m4t for inter weights)
                v.tensor_tensor(out=e01t[:P], in0=Tn[0], in1=Tn[1], op=ALU.is_equal)
                v.tensor_tensor(out=eB[:P], in0=Tn[0], in1=Tn[2], op=ALU.is_equal)
                v.tensor_tensor(out=eC[:P], in0=Tn[1], in1=Tn[2], op=ALU.is_equal)
                v.tensor_tensor(out=m3t[:P], in0=eB[:P], in1=eC[:P], op=ALU.max)
                v.tensor_tensor(out=eB[:P], in0=Tn[0], in1=Tn[3], op=ALU.is_equal)
                v.tensor_tensor(out=eC[:P], in0=Tn[1], in1=Tn[3], op=ALU.is_equal)
                v.tensor_tensor(out=eD[:P], in0=eB[:P], in1=eC[:P], op=ALU.max)
                v.tensor_tensor(out=eB[:P], in0=Tn[2], in1=Tn[3], op=ALU.is_equal)
                v.tensor_tensor(out=m4t[:P], in0=eD[:P], in1=eB[:P], op=ALU.max)
                v.tensor_tensor(out=eC[:P], in0=e01t[:P], in1=m3t[:P], op=ALU.add)
                v.tensor_tensor(out=msumT[:P], in0=eC[:P], in1=m4t[:P], op=ALU.add)

                # ---- E: inter = |Pset cap Tset| ----
                wts = [None, e01t, m3t, m4t]
                Sh_cur, Sh_nxt = Sh1, Sh2
                Sw_cur, Sw_nxt = Sw1, Sw2
                for k in range(4):
                    v.tensor_tensor(out=eA[:P], in0=Pn[0], in1=Tn[k], op=ALU.is_equal)
                    v.tensor_tensor(out=eB[:P], in0=Pn[1], in1=Tn[k], op=ALU.is_equal)
                    v.tensor_tensor(out=eC[:P], in0=Pn[2], in1=Tn[k], op=ALU.is_equal)
                    v.tensor_tensor(out=eD[:P], in0=Pn[3], in1=Tn[k], op=ALU.is_equal)
                    v.tensor_tensor(out=eE[:P], in0=eA[:P], in1=eB[:P], op=ALU.max)
                    v.tensor_tensor(out=eA[:P], in0=eC[:P], in1=eD[:P], op=ALU.max)
                    if k == 0:
                        v.tensor_tensor(out=Sh_cur[:P], in0=eE[:P], in1=eA[:P], op=ALU.max)  # h0
                    else:
                        v.tensor_tensor(out=hk[:P], in0=eE[:P], in1=eA[:P], op=ALU.max)
                        v.tensor_tensor(out=Sh_nxt[:P], in0=Sh_cur[:P], in1=hk[:P], op=ALU.add)
                        Sh_cur, Sh_nxt = Sh_nxt, Sh_cur
                        if k == 1:
                            v.tensor_tensor(out=Sw_cur[:P], in0=hk[:P], in1=wts[k][:P], op=ALU.mult)
                        else:
                            v.tensor_tensor(out=pw[:P], in0=hk[:P], in1=wts[k][:P], op=ALU.mult)
                            v.tensor_tensor(out=Sw_nxt[:P], in0=Sw_cur[:P], in1=pw[:P], op=ALU.add)
                            Sw_cur, Sw_nxt = Sw_nxt, Sw_cur
                v.tensor_tensor(out=inter[:P], in0=Sh_cur[:P], in1=Sw_cur[:P], op=ALU.subtract)

                # ---- B: per-class presence M_c = OR_i (pc_i == c) ----
                # z tiles come from ScalarE: z = relu(1 - (pc - c)^2)
                zv_ = [zb[:P].rearrange("p (f i) -> p f i", i=4) for zb in (z, z2)]
                for ci, c in enumerate(_CLASSES):
                    zv = zv_[ci % 2]
                    v.wait_ge(s_z, 10 * ipass + ci + 1)
                    v.tensor_tensor(out=sc1[:P], in0=zv[:, :, 0], in1=zv[:, :, 1], op=ALU.max)
                    v.tensor_tensor(out=sc2[:P], in0=zv[:, :, 2], in1=zv[:, :, 3], op=ALU.max)
                    v.tensor_tensor(out=M[c][:P], in0=sc1[:P], in1=sc2[:P],
                                    op=ALU.max).then_inc(s_zread, 1)

                # ---- C: template intersections I_tau ----
                v.tensor_tensor(out=sc1[:P], in0=M[0][:P], in1=M[7][:P], op=ALU.add)   # 0+7
                v.tensor_tensor(out=I6[0][:P], in0=sc1[:P], in1=M[4][:P], op=ALU.add)  # I(0,4,7)
                v.tensor_tensor(out=I6[3][:P], in0=sc1[:P], in1=M[3][:P], op=ALU.add)  # i(0,3,7)
                v.tensor_tensor(out=sc2[:P], in0=M[5][:P], in1=M[0][:P], op=ALU.add)   # 5+0
                v.tensor_tensor(out=I6[1][:P], in0=sc2[:P], in1=M[9][:P], op=ALU.add)  # IV(5,9,0)
                v.tensor_tensor(out=I6[4][:P], in0=sc2[:P], in1=M[8][:P], op=ALU.add)  # iv(5,8,0)
                v.tensor_tensor(out=sc3[:P], in0=M[7][:P], in1=M[2][:P], op=ALU.add)   # 7+2
                v.tensor_tensor(out=I6[2][:P], in0=sc3[:P], in1=M[11][:P], op=ALU.add) # V(7,11,2)
                v.tensor_tensor(out=I6[5][:P], in0=sc3[:P], in1=M[10][:P], op=ALU.add) # v(7,10,2)

                # ---- F: similarity ----
                CNUM = 8.0 * _EPS + 12.0 * _EPS * _EPS
                v.tensor_tensor(out=sc1[:P], in0=msumP[:P], in1=msumT[:P], op=ALU.add)   # su
                v.scalar_tensor_tensor(out=sc2[:P], in0=sc1[:P], scalar=-_EPS,
                                       in1=inter[:P], op0=ALU.mult, op1=ALU.add)          # nume
                v.tensor_scalar(out=sc3[:P], in0=msumP[:P], scalar1=-1.0,
                                scalar2=4.0 + 12.0 * _EPS, op0=ALU.mult, op1=ALU.add)
                v.tensor_scalar(out=sc4[:P], in0=msumT[:P], scalar1=-1.0,
                                scalar2=4.0 + 12.0 * _EPS, op0=ALU.mult, op1=ALU.add)
                v.tensor_tensor(out=rp[:P], in0=sc3[:P], in1=sc4[:P], op=ALU.mult)
                v.reciprocal(out=sc5[:P], in_=rp[:P])
                v.scalar_tensor_tensor(out=sc3[:P, :W], in0=sc2[:P, :W], scalar=CNUM,
                                       in1=sc5[:P, :W], op0=ALU.add, op1=ALU.mult,
                                       accum_out=sim_acc[ipass][:P])

                # ---- G: q_tau = I_tau / (s_p + 3 - I_tau), two per reciprocal ----
                gu2, gt = rp, rt  # more dead-scratch aliases
                for j in range(3):
                    j2 = j + 3
                    v.tensor_tensor(out=gw[:P], in0=I6[j][:P], in1=msumP[:P], op=ALU.add)
                    v.tensor_scalar(out=gu[:P], in0=gw[:P], scalar1=-1.0, scalar2=7.0,
                                    op0=ALU.mult, op1=ALU.add)
                    v.tensor_tensor(out=gw[:P], in0=I6[j2][:P], in1=msumP[:P], op=ALU.add)
                    v.tensor_scalar(out=gu2[:P], in0=gw[:P], scalar1=-1.0, scalar2=7.0,
                                    op0=ALU.mult, op1=ALU.add)
                    v.tensor_tensor(out=gt[:P], in0=gu[:P], in1=gu2[:P], op=ALU.mult)
                    v.reciprocal(out=gg[:P], in_=gt[:P])
                    v.tensor_tensor(out=gt[:P], in0=I6[j][:P], in1=gu2[:P], op=ALU.mult)
                    v.tensor_tensor(out=q6[j][:P], in0=gt[:P], in1=gg[:P], op=ALU.mult)
                    v.tensor_tensor(out=gt[:P], in0=I6[j2][:P], in1=gu[:P], op=ALU.mult)
                    v.tensor_tensor(out=q6[j2][:P], in0=gt[:P], in1=gg[:P], op=ALU.mult)

                # ---- H: window terms ----
                def win(qs, a, b, dst):
                    v.tensor_tensor(out=a[:P, 0:W], in0=qs[0][:P, 0:W], in1=qs[1][:P, 1:W + 1], op=ALU.add)
                    v.tensor_tensor(out=b[:P, 0:W], in0=a[:P, 0:W], in1=qs[2][:P, 2:W + 2], op=ALU.add)
                    v.tensor_tensor(out=dst[:P, 0:W], in0=b[:P, 0:W], in1=qs[0][:P, 3:W + 3], op=ALU.add)

                win([q6[0], q6[1], q6[2]], Qa, Qb, QM)
                win([q6[3], q6[4], q6[5]], Qc, Qd, Qm)
                v.tensor_tensor(out=sc1[:P, 0:W], in0=QM[:P, 0:W], in1=Qm[:P, 0:W], op=ALU.max)
                v.tensor_scalar(out=sc2[:P, :W], in0=sc1[:P, :W], scalar1=1.0,
                                scalar2=None, op0=ALU.mult, op1=ALU.add,
                                accum_out=win_acc[ipass][:P])

            # ---- final accumulation across passes ----
            v.tensor_tensor(out=tot[0][:P], in0=sim_acc[0][:P], in1=sim_acc[1][:P], op=ALU.add)
            v.tensor_tensor(out=tot[1][:P], in0=win_acc[0][:P], in1=win_acc[1][:P],
                            op=ALU.add).then_inc(v_sem, 1)

        @block.scalar
        def _(a):
            AF = mybir.ActivationFunctionType
            for ipass in range(2):
                a.wait_ge(s_pc, ipass + 1)
                for ci, c in enumerate(_CLASSES):
                    zb = (z, z2)[ci % 2]
                    if 10 * ipass + ci >= 2:
                        # DVE must have consumed the tile we are overwriting
                        a.wait_ge(s_zread, 10 * ipass + ci - 1)
                    a.activation(out=zsq[:P], in_=pcp[:P], func=AF.Square,
                                 bias=-float(c), scale=1.0)
                    a.activation(out=zb[:P], in_=zsq[:P], func=AF.Relu,
                                 bias=1.0, scale=-1.0).then_inc(s_z, 1)

        @block.gpsimd
        def _(g):
            from concourse.bass import AP as _AP
            g.wait_ge(v_sem, 1)
            g.dma_start(out=_AP(out_d, 0, [[1, P]]), in_=tot[0][:P]).then_inc(odma_sem, 16)
            g.dma_start(out=_AP(out_d, P, [[1, P]]), in_=tot[1][:P]).then_inc(odma_sem, 16)
            g.wait_ge(odma_sem, 32)

    return nc


# ---------------------------------------------------------------------------
# host side
# ---------------------------------------------------------------------------

def _host_window_terms(rows_p, rows_t):
    """max(QM, Qm) for windows starting at offset 0..len-4 of the given rows
    (float64, mirrors the device math). rows_*: [n, 4] float arrays."""
    pc_p = np.mod(rows_p.astype(np.float64), 12.0)
    n = pc_p.shape[0]

    def presence(pc_row, c):
        return 1.0 if np.any(pc_row == c) else 0.0

    out = []
    for i in range(n - 3):
        QM = 0.0
        Qm = 0.0
        for off, tj in ((0, 0), (1, 1), (2, 2), (3, 0)):
            row = pc_p[i + off]
            sp = len(set(row.tolist()))
            for base, acc in ((0, "M"), (3, "m")):
                tpl = _TEMPLATES[base + tj]
                I = sum(presence(row, c) for c in tpl)
                q = I / (sp + 3.0 - I)
                if acc == "M":
                    QM += q
                else:
                    Qm += q
        out.append(max(QM, Qm))
    return np.array(out)


_NC_CACHE = {}


def _get_nc():
    if "nc" not in _NC_CACHE:
        _NC_CACHE["nc"] = build_nc()
    return _NC_CACHE["nc"]


def _make_in_maps(preds, targs):
    preds = np.ascontiguousarray(np.asarray(preds, dtype=np.float32))
    targs = np.ascontiguousarray(np.asarray(targs, dtype=np.float32))
    assert preds.shape == (_T, 4) and targs.shape == (_T, 4)
    n_in = (_P - 1) * _S + _F  # 125003 rows per core
    pad = np.zeros((3, 4), np.float32)
    pred_pad = np.concatenate([preds, pad], axis=0)
    targ_pad = np.concatenate([targs, pad], axis=0)
    in_maps = []
    for c in range(_NCORES):
        s0 = c * _PERCORE
        in_maps.append({
            "xp": np.ascontiguousarray(pred_pad[s0:s0 + n_in]).reshape(-1),
            "xt": np.ascontiguousarray(targ_pad[s0:s0 + n_in]).reshape(-1),
        })
    return in_maps


def kernel(chord_predictions: np.ndarray, chord_targets: np.ndarray) -> np.ndarray:
    from concourse.bass_utils import run_bass_kernel_spmd

    preds = np.asarray(chord_predictions, dtype=np.float32)
    targs = np.asarray(chord_targets, dtype=np.float32)
    T = _T
    in_maps = _make_in_maps(preds, targs)
    pred_pad = np.concatenate([preds, np.zeros((3, 4), np.float32)], axis=0)
    targ_pad = np.concatenate([targs, np.zeros((3, 4), np.float32)], axis=0)

    nc = _get_nc()
    res = run_bass_kernel_spmd(nc, in_maps, list(range(_NCORES)))

    sim_sum = 0.0
    win_sum = 0.0
    for c in range(_NCORES):
        o = np.asarray(res.results[c]["out"], dtype=np.float64)
        sim_sum += o[0].sum()
        win_sum += o[1].sum()

    # Remove the 3 invalid windows (starts T-3..T-1, computed from zero pad)
    tail = _host_window_terms(pred_pad[T - 3: T + 3], targ_pad[T - 3: T + 3])
    win_sum -= tail.sum()

    n_win = T - 3
    similarity_loss = 1.0 - sim_sum / T
    progression_penalty = 1.0 - win_sum / (4.0 * n_win)
    loss = similarity_loss + 0.5 * progression_penalty
    return np.float32(loss)


# revision 16
# speedup vs baseline: 1.0016x; 1.0016x over previous
"""ChordProgressionLoss Trainium2 kernel.

Strategy (pure data parallel over the time axis, 8 cores):
  - T = 1,000,000 rows of 4 notes (pred + targ). Each core handles 125,000
    consecutive "positions" (timesteps / window starts).
  - Per core, time is laid out as P=125 SBUF partitions x S=1000 positions,
    each partition holding a contiguous chunk of F=S+3 rows (3-row halo so
    the 4-step progression windows never cross a partition).
  - Per row the kernel computes, with exact small-integer arithmetic:
      pc       = note mod 12 (rounding-agnostic int-convert trick)
      M_c      = presence of pitch-class c (10 classes used by templates)
      s_p, s_t = distinct-class counts via pairwise-equality dedup
      inter    = |Pset cap Tset| via pairwise equality + dedup weights
      I_tau    = |Pset cap template| for the 6 distinct template chords
      q_tau    = I_tau / (s_p + 3 - I_tau)  (= 1 - Jaccard distance)
      sim      = (inter + eps*(s_p+s_t) + 12 eps^2) / ((s_p+12eps)(s_t+12eps))
  - Window terms: min(maj, mino) = 1 - max(QM, Qm)/4 where
      QM[i] = qM1[i] + qM2[i+1] + qM3[i+2] + qM1[i+3]  (same for minor),
    accumulated per partition with on-instruction accumulators.
  - Device returns per-partition sums [125, 2] (sim sum, window-max sum);
    the host reduces across partitions/cores in float64 and fixes up the
    3 windows that fall off the end of the sequence (they are computed from
    zero-padded rows on core 7 and subtracted here).
"""

import sys

sys.path.insert(0, "/opt/trn_rl_repo")

import numpy as np

_NCORES = 8
_T = 1_000_000
_PERCORE = _T // _NCORES  # 125000

# Device geometry (full problem)
_P = 125      # partitions used
_S = 1000     # positions per partition
_F = _S + 3   # rows per partition (halo 3)

_EPS = 1.0e-6

# 6 distinct template chords (major I, IV, V; minor i, iv, v)
_TEMPLATES = [
    (0, 4, 7), (5, 9, 0), (7, 11, 2),   # major
    (0, 3, 7), (5, 8, 0), (7, 10, 2),   # minor
]
_CLASSES = sorted({c for t in _TEMPLATES for c in t})  # 10 classes


def build_nc(P=_P, S=_S):
    """Build the per-core Bass program (SPMD: same program on all cores)."""
    from concourse import bass, mybir
    from concourse.bass import AP

    F32 = mybir.dt.float32
    I32 = mybir.dt.int32
    ALU = mybir.AluOpType

    F = S + 3
    assert S % 2 == 0
    W = S // 2          # window positions per pass
    L = W + 3           # rows needed per pass
    n_in = (P - 1) * S + F  # rows per core

    nc = bass.Bass("TRN2", target_bir_lowering=False, debug=False,
                   detect_race_conditions=False)
    # const bias APs for the ScalarE activations (pattern from Bass.__init__)
    for _val in sorted({-float(c) for c in _CLASSES if c} | {1.0}):
        if (F32, _val) in nc.const_aps.aps:
            continue
        _ct = nc.alloc_sbuf_tensor(f"const-float32-{_val}", [128, 1], F32)
        nc.gpsimd.memset(_ct.ap(), _val)
        nc.const_aps.aps[(F32, _val)] = _ct.ap()
    nc.all_engine_barrier()
    xp_d = nc.declare_dram_parameter("xp", [n_in * 4], F32, isOutput=False)
    xt_d = nc.declare_dram_parameter("xt", [n_in * 4], F32, isOutput=False)
    out_d = nc.declare_dram_parameter("out", [2, P], F32, isOutput=True)

    sb = {}

    def t4(nm, dt=F32):  # [P, 4F] full-chunk tile
        sb[nm] = nc.alloc_sbuf_tensor(nm, [128, 4 * F], dt)
        return sb[nm].ap()

    def tL(nm, dt=F32):  # [P, L] per-pass series tile
        sb[nm] = nc.alloc_sbuf_tensor(nm, [128, L], dt)
        return sb[nm].ap()

    def t1(nm, dt=F32):  # [P, 1] accumulator
        sb[nm] = nc.alloc_sbuf_tensor(nm, [128, 1], dt)
        return sb[nm].ap()

    sb["x2"] = nc.alloc_sbuf_tensor("x2", [128, 8 * F], F32)
    x2 = sb["x2"].ap()
    xp, xt = x2[:, 0:4 * F], x2[:, 4 * F:8 * F]

    def tz(nm, dt=F32):  # [P, 4L] per-pass chunk tile
        sb[nm] = nc.alloc_sbuf_tensor(nm, [128, 4 * L], dt)
        return sb[nm].ap()

    sb["pc2"] = nc.alloc_sbuf_tensor("pc2", [128, 8 * L], F32)
    pc2 = sb["pc2"].ap()
    pcp, pct = pc2[:, 0:4 * L], pc2[:, 4 * L:8 * L]
    sb["ii"] = nc.alloc_sbuf_tensor("ii", [128, 8 * L], I32)
    ii = sb["ii"].ap()
    ng = ii.bitcast(F32)   # ii is dead once rr exists; reuse as f32 scratch
    sb["rr"] = nc.alloc_sbuf_tensor("rr", [128, 8 * L], F32)
    rr = sb["rr"].ap()
    z = tz("z")
    z2 = tz("z2")
    zsq = tz("zsq")

    M = {c: tL(f"M{c}") for c in _CLASSES}
    I6 = [tL(f"I{j}") for j in range(6)]
    sc1, sc2, sc3, sc4, sc5 = (tL(f"sc{k}") for k in range(1, 6))
    eA, eB, eC, eD, eE = (tL(f"e{k}") for k in "ABCDE")
    e01t, m3t, m4t = tL("e01t"), tL("m3t"), tL("m4t")
    msumP, msumT = tL("msumP"), tL("msumT")
    Sh1, Sh2, Sw1, Sw2, hk, pw = (tL(nm) for nm in
                                  ["Sh1", "Sh2", "Sw1", "Sw2", "hk", "pw"])
    inter = tL("inter")
    # aliases into buffers that are dead by the time these are used
    rp, rt = hk, pw            # F-stage: E scratch is free
    gw, gu, gg = Sh1, Sh2, Sw1  # G-stage: E accumulators are free
    q6 = [tL(f"q{j}") for j in range(6)]
    QM, Qm = tL("QM"), tL("Qm")
    Qa, Qb, Qc, Qd = eA, eB, eC, eD  # H-stage: compare scratch is free

    sim_acc = [t1("sim_a0"), t1("sim_a1")]
    win_acc = [t1("win_a0"), t1("win_a1")]
    tot = t1("tot_sim"), t1("tot_win")

    with (
        nc.Block() as block,
        nc.semaphore("dma_sem") as dma_sem,
        nc.semaphore("dma2_sem") as dma2_sem,
        nc.semaphore("odma_sem") as odma_sem,
        nc.semaphore("s_pc") as s_pc,
        nc.semaphore("s_z") as s_z,
        nc.semaphore("s_zread") as s_zread,
        nc.semaphore("v_sem") as v_sem,
    ):

        @block.sync
        def _(s):
            c1 = 4 * L                    # columns needed by pass 0
            s.dma_start(out=x2[:P, :c1], in_=AP(xp_d, 0, [[4 * S, P], [1, c1]])).then_inc(dma_sem, 16)
            s.dma_start(out=x2[:P, 4 * F:4 * F + c1], in_=AP(xt_d, 0, [[4 * S, P], [1, c1]])).then_inc(dma_sem, 16)
            s.dma_start(out=x2[:P, c1:4 * F], in_=AP(xp_d, c1, [[4 * S, P], [1, 4 * F - c1]])).then_inc(dma2_sem, 16)
            s.dma_start(out=x2[:P, 4 * F + c1:], in_=AP(xt_d, c1, [[4 * S, P], [1, 4 * F - c1]])).then_inc(dma2_sem, 16)

        @block.vector
        def _(v):
            v.wait_ge(dma_sem, 32)

            for ipass, fs in enumerate((0, W)):
                if ipass == 1:
                    v.wait_ge(dma2_sem, 32)
                # ---- stage A: pc = note mod 12, pred+targ in one chain ----
                # [P, 2, 4L] view: both tensors' pass columns in one op
                slq = AP(sb["x2"], 4 * fs, [[8 * F, P], [4 * F, 2], [1, 4 * L]])
                pcq = pc2[:P].rearrange("p (h c) -> p h c", h=2)
                iiq = ii[:P].rearrange("p (h c) -> p h c", h=2)
                rrq = rr[:P].rearrange("p (h c) -> p h c", h=2)
                ngq = ng[:P].rearrange("p (h c) -> p h c", h=2)
                v.tensor_scalar(out=iiq, in0=slq, scalar1=float(1.0 / 12.0),
                                scalar2=0.0, op0=ALU.mult, op1=ALU.add)
                v.scalar_tensor_tensor(out=rrq, in0=iiq, scalar=-12.0,
                                       in1=slq, op0=ALU.mult, op1=ALU.add)
                v.tensor_single_scalar(out=ngq, in_=rrq, scalar=0.0, op=ALU.is_lt)
                v.scalar_tensor_tensor(out=pcq, in0=ngq, scalar=12.0,
                                       in1=rrq, op0=ALU.mult,
                                       op1=ALU.add).then_inc(s_pc, 1)

                pc4p = pcp[:P]       # [P, 4L] contiguous
                pc4t = pct[:P]
                pvp = pc4p.rearrange("p (f i) -> p f i", i=4)
                pvt = pc4t.rearrange("p (f i) -> p f i", i=4)
                Pn = [pvp[:, :, i] for i in range(4)]
                Tn = [pvt[:, :, i] for i in range(4)]

                # ---- D: dedup msum for pred and targ ----
                # pred
                v.tensor_tensor(out=eA[:P], in0=Pn[0], in1=Pn[1], op=ALU.is_equal)  # e01
                v.tensor_tensor(out=eB[:P], in0=Pn[0], in1=Pn[2], op=ALU.is_equal)  # e02
                v.tensor_tensor(out=eC[:P], in0=Pn[1], in1=Pn[2], op=ALU.is_equal)  # e12
                v.tensor_tensor(out=eD[:P], in0=eB[:P], in1=eC[:P], op=ALU.max)     # m3
                v.tensor_tensor(out=eB[:P], in0=Pn[0], in1=Pn[3], op=ALU.is_equal)  # e03
                v.tensor_tensor(out=eC[:P], in0=Pn[1], in1=Pn[3], op=ALU.is_equal)  # e13
                v.tensor_tensor(out=eE[:P], in0=eB[:P], in1=eC[:P], op=ALU.max)     # m4a
                v.tensor_tensor(out=eB[:P], in0=Pn[2], in1=Pn[3], op=ALU.is_equal)  # e23
                v.tensor_tensor(out=eC[:P], in0=eE[:P], in1=eB[:P], op=ALU.max)     # m4
                v.tensor_tensor(out=eE[:P], in0=eA[:P], in1=eD[:P], op=ALU.add)     # e01+m3
                v.tensor_tensor(out=msumP[:P], in0=eE[:P], in1=eC[:P], op=ALU.add)
                # targ (keep e01t, m3t, m4t for inter weights)
                v.tensor_tensor(out=e01t[:P], in0=Tn[0], in1=Tn[1], op=ALU.is_equal)
                v.tensor_tensor(out=eB[:P], in0=Tn[0], in1=Tn[2], op=ALU.is_equal)
                v.tensor_tensor(out=eC[:P], in0=Tn[1], in1=Tn[2], op=ALU.is_equal)
                v.tensor_tensor(out=m3t[:P], in0=eB[:P], in1=eC[:P], op=ALU.max)
                v.tensor_tensor(out=eB[:P], in0=Tn[0], in1=Tn[3], op=ALU.is_equal)
                v.tensor_tensor(out=eC[:P], in0=Tn[1], in1=Tn[3], op=ALU.is_equal)
                v.tensor_tensor(out=eD[:P], in0=eB[:P], in1=eC[:P], op=ALU.max)
                v.tensor_tensor(out=eB[:P], in0=Tn[2], in1=Tn[3], op=ALU.is_equal)
                v.tensor_tensor(out=m4t[:P], in0=eD[:P], in1=eB[:P], op=ALU.max)
                v.tensor_tensor(out=eC[:P], in0=e01t[:P], in1=m3t[:P], op=ALU.add)
                v.tensor_tensor(out=msumT[:P], in0=eC[:P], in1=m4t[:P], op=ALU.add)

                # ---- E: inter = |Pset cap Tset| ----
                wts = [None, e01t, m3t, m4t]
                Sh_cur, Sh_nxt = Sh1, Sh2
                Sw_cur, Sw_nxt = Sw1, Sw2
                for k in range(4):
                    v.tensor_tensor(out=eA[:P], in0=Pn[0], in1=Tn[k], op=ALU.is_equal)
                    v.tensor_tensor(out=eB[:P], in0=Pn[1], in1=Tn[k], op=ALU.is_equal)
                    v.tensor_tensor(out=eC[:P], in0=Pn[2], in1=Tn[k], op=ALU.is_equal)
                    v.tensor_tensor(out=eD[:P], in0=Pn[3], in1=Tn[k], op=ALU.is_equal)
                    v.tensor_tensor(out=eE[:P], in0=eA[:P], in1=eB[:P], op=ALU.max)
                    v.tensor_tensor(out=eA[:P], in0=eC[:P], in1=eD[:P], op=ALU.max)
                    if k == 0:
                        v.tensor_tensor(out=Sh_cur[:P], in0=eE[:P], in1=eA[:P], op=ALU.max)  # h0
                    else:
                        v.tensor_tensor(out=hk[:P], in0=eE[:P], in1=eA[:P], op=ALU.max)
                        v.tensor_tensor(out=Sh_nxt[:P], in0=Sh_cur[:P], in1=hk[:P], op=ALU.add)
                        Sh_cur, Sh_nxt = Sh_nxt, Sh_cur
                        if k == 1:
                            v.tensor_tensor(out=Sw_cur[:P], in0=hk[:P], in1=wts[k][:P], op=ALU.mult)
                        else:
                            v.tensor_tensor(out=pw[:P], in0=hk[:P], in1=wts[k][:P], op=ALU.mult)
                            v.tensor_tensor(out=Sw_nxt[:P], in0=Sw_cur[:P], in1=pw[:P], op=ALU.add)
                            Sw_cur, Sw_nxt = Sw_nxt, Sw_cur
                v.tensor_tensor(out=inter[:P], in0=Sh_cur[:P], in1=Sw_cur[:P], op=ALU.subtract)

                # ---- B: per-class presence M_c = OR_i (pc_i == c) ----
                # z tiles come from ScalarE: z = relu(1 - (pc - c)^2)
                zv_ = [zb[:P].rearrange("p (f i) -> p f i", i=4) for zb in (z, z2)]
                for ci, c in enumerate(_CLASSES):
                    zv = zv_[ci % 2]
                    v.wait_ge(s_z, 10 * ipass + ci + 1)
                    v.tensor_tensor(out=sc1[:P], in0=zv[:, :, 0], in1=zv[:, :, 1], op=ALU.max)
                    v.tensor_tensor(out=sc2[:P], in0=zv[:, :, 2], in1=zv[:, :, 3], op=ALU.max)
                    v.tensor_tensor(out=M[c][:P], in0=sc1[:P], in1=sc2[:P],
                                    op=ALU.max).then_inc(s_zread, 1)

                # ---- C: template intersections I_tau ----
                v.tensor_tensor(out=sc1[:P], in0=M[0][:P], in1=M[7][:P], op=ALU.add)   # 0+7
                v.tensor_tensor(out=I6[0][:P], in0=sc1[:P], in1=M[4][:P], op=ALU.add)  # I(0,4,7)
                v.tensor_tensor(out=I6[3][:P], in0=sc1[:P], in1=M[3][:P], op=ALU.add)  # i(0,3,7)
                v.tensor_tensor(out=sc2[:P], in0=M[5][:P], in1=M[0][:P], op=ALU.add)   # 5+0
                v.tensor_tensor(out=I6[1][:P], in0=sc2[:P], in1=M[9][:P], op=ALU.add)  # IV(5,9,0)
                v.tensor_tensor(out=I6[4][:P], in0=sc2[:P], in1=M[8][:P], op=ALU.add)  # iv(5,8,0)
                v.tensor_tensor(out=sc3[:P], in0=M[7][:P], in1=M[2][:P], op=ALU.add)   # 7+2
                v.tensor_tensor(out=I6[2][:P], in0=sc3[:P], in1=M[11][:P], op=ALU.add) # V(7,11,2)
                v.tensor_tensor(out=I6[5][:P], in0=sc3[:P], in1=M[10][:P], op=ALU.add) # v(7,10,2)

                # ---- F: similarity ----
                CNUM = 8.0 * _EPS + 12.0 * _EPS * _EPS
                v.tensor_tensor(out=sc1[:P], in0=msumP[:P], in1=msumT[:P], op=ALU.add)   # su
                v.scalar_tensor_tensor(out=sc2[:P], in0=sc1[:P], scalar=-_EPS,
                                       in1=inter[:P], op0=ALU.mult, op1=ALU.add)          # nume
                v.tensor_scalar(out=sc3[:P], in0=msumP[:P], scalar1=-1.0,
                                scalar2=4.0 + 12.0 * _EPS, op0=ALU.mult, op1=ALU.add)
                v.tensor_scalar(out=sc4[:P], in0=msumT[:P], scalar1=-1.0,
                                scalar2=4.0 + 12.0 * _EPS, op0=ALU.mult, op1=ALU.add)
                v.tensor_tensor(out=rp[:P], in0=sc3[:P], in1=sc4[:P], op=ALU.mult)
                v.reciprocal(out=sc5[:P], in_=rp[:P])
                v.scalar_tensor_tensor(out=sc3[:P, :W], in0=sc2[:P, :W], scalar=CNUM,
                                       in1=sc5[:P, :W], op0=ALU.add, op1=ALU.mult,
                                       accum_out=sim_acc[ipass][:P])

                # ---- G: q_tau = I_tau / (s_p + 3 - I_tau), two per reciprocal ----
                gu2, gt = rp, rt  # more dead-scratch aliases
                for j in range(3):
                    j2 = j + 3
                    v.tensor_tensor(out=gw[:P], in0=I6[j][:P], in1=msumP[:P], op=ALU.add)
                    v.tensor_scalar(out=gu[:P], in0=gw[:P], scalar1=-1.0, scalar2=7.0,
                                    op0=ALU.mult, op1=ALU.add)
                    v.tensor_tensor(out=gw[:P], in0=I6[j2][:P], in1=msumP[:P], op=ALU.add)
                    v.tensor_scalar(out=gu2[:P], in0=gw[:P], scalar1=-1.0, scalar2=7.0,
                                    op0=ALU.mult, op1=ALU.add)
                    v.tensor_tensor(out=gt[:P], in0=gu[:P], in1=gu2[:P], op=ALU.mult)
                    v.reciprocal(out=gg[:P], in_=gt[:P])
                    v.tensor_tensor(out=gt[:P], in0=I6[j][:P], in1=gu2[:P], op=ALU.mult)
                    v.tensor_tensor(out=q6[j][:P], in0=gt[:P], in1=gg[:P], op=ALU.mult)
                    v.tensor_tensor(out=gt[:P], in0=I6[j2][:P], in1=gu[:P], op=ALU.mult)
                    v.tensor_tensor(out=q6[j2][:P], in0=gt[:P], in1=gg[:P], op=ALU.mult)

                # ---- H: window terms ----
                def win(qs, a, b, dst):
                    v.tensor_tensor(out=a[:P, 0:W], in0=qs[0][:P, 0:W], in1=qs[1][:P, 1:W + 1], op=ALU.add)
                    v.tensor_tensor(out=b[:P, 0:W], in0=a[:P, 0:W], in1=qs[2][:P, 2:W + 2], op=ALU.add)
                    v.tensor_tensor(out=dst[:P, 0:W], in0=b[:P, 0:W], in1=qs[0][:P, 3:W + 3], op=ALU.add)

                win([q6[0], q6[1], q6[2]], Qa, Qb, QM)
                win([q6[3], q6[4], q6[5]], Qc, Qd, Qm)
                v.tensor_tensor(out=sc1[:P, 0:W], in0=QM[:P, 0:W], in1=Qm[:P, 0:W], op=ALU.max)
                v.tensor_scalar(out=sc2[:P, :W], in0=sc1[:P, :W], scalar1=1.0,
                                scalar2=None, op0=ALU.mult, op1=ALU.add,
                                accum_out=win_acc[ipass][:P])

            # ---- final accumulation across passes ----
            v.tensor_tensor(out=tot[0][:P], in0=sim_acc[0][:P], in1=sim_acc[1][:P], op=ALU.add)
            v.tensor_tensor(out=tot[1][:P], in0=win_acc[0][:P], in1=win_acc[1][:P],
                            op=ALU.add).then_inc(v_sem, 1)

        @block.scalar
        def _(a):
            AF = mybir.ActivationFunctionType
            for ipass in range(2):
                a.wait_ge(s_pc, ipass + 1)
                for ci, c in enumerate(_CLASSES):
                    zb = (z, z2)[ci % 2]
                    if 10 * ipass + ci >= 2:
                        # DVE must have consumed the tile we are overwriting
                        a.wait_ge(s_zread, 10 * ipass + ci - 1)
                    a.activation(out=zsq[:P], in_=pcp[:P], func=AF.Square,
                                 bias=-float(c), scale=1.0)
                    a.activation(out=zb[:P], in_=zsq[:P], func=AF.Relu,
                                 bias=1.0, scale=-1.0).then_inc(s_z, 1)

        @block.gpsimd
        def _(g):
            from concourse.bass import AP as _AP
            g.wait_ge(v_sem, 1)
            g.dma_start(out=_AP(out_d, 0, [[1, P]]), in_=tot[0][:P]).then_inc(odma_sem, 16)
            g.dma_start(out=_AP(out_d, P, [[1, P]]), in_=tot[1][:P]).then_inc(odma_sem, 16)
            g.wait_ge(odma_sem, 32)

    return nc


# ---------------------------------------------------------------------------
# host side
# ---------------------------------------------------------------------------

def _host_window_terms(rows_p, rows_t):
    """max(QM, Qm) for windows starting at offset 0..len-4 of the given rows
    (float64, mirrors the device math). rows_*: [n, 4] float arrays."""
    pc_p = np.mod(rows_p.astype(np.float64), 12.0)
    n = pc_p.shape[0]

    def presence(pc_row, c):
        return 1.0 if np.any(pc_row == c) else 0.0

    out = []
    for i in range(n - 3):
        QM = 0.0
        Qm = 0.0
        for off, tj in ((0, 0), (1, 1), (2, 2), (3, 0)):
            row = pc_p[i + off]
            sp = len(set(row.tolist()))
            for base, acc in ((0, "M"), (3, "m")):
                tpl = _TEMPLATES[base + tj]
                I = sum(presence(row, c) for c in tpl)
                q = I / (sp + 3.0 - I)
                if acc == "M":
                    QM += q
                else:
                    Qm += q
        out.append(max(QM, Qm))
    return np.array(out)


_NC_CACHE = {}


def _get_nc():
    if "nc" not in _NC_CACHE:
        _NC_CACHE["nc"] = build_nc()
    return _NC_CACHE["nc"]


def _make_in_maps(preds, targs):
    preds = np.ascontiguousarray(np.asarray(preds, dtype=np.float32))
    targs = np.ascontiguousarray(np.asarray(targs, dtype=np.float32))
    assert preds.shape == (_T, 4) and targs.shape == (_T, 4)
    n_in = (_P - 1) * _S + _F  # 125003 rows per core
    pad = np.zeros((3, 4), np.float32)
    pred_pad = np.concatenate([preds, pad], axis=0)
    targ_pad = np.concatenate([targs, pad], axis=0)
    in_maps = []
    for c in range(_NCORES):
        s0 = c * _PERCORE
        in_maps.append({
            "xp": np.ascontiguousarray(pred_pad[s0:s0 + n_in]).reshape(-1),
            "xt": np.ascontiguousarray(targ_pad[s0:s0 + n_in]).reshape(-1),
        })
    return in_maps


def kernel(chord_predictions: np.ndarray, chord_targets: np.ndarray) -> np.ndarray:
    from concourse.bass_utils import run_bass_kernel_spmd

    preds = np.asarray(chord_predictions, dtype=np.float32)
    targs = np.asarray(chord_targets, dtype=np.float32)
    T = _T
    in_maps = _make_in_maps(preds, targs)
    pred_pad = np.concatenate([preds, np.zeros((3, 4), np.float32)], axis=0)
    targ_pad = np.concatenate([targs, np.zeros((3, 4), np.float32)], axis=0)

    nc = _get_nc()
    res = run_bass_kernel_spmd(nc, in_maps, list(range(_NCORES)))

    sim_sum = 0.0
    win_sum = 0.0
    for c in range(_NCORES):
        o = np.asarray(res.results[c]["out"], dtype=np.float64)
        sim_sum += o[0].sum()
        win_sum += o[1].sum()

    # Remove the 3 invalid windows (starts T-3..T-1, computed from zero pad)
    tail = _host_window_terms(pred_pad[T - 3: T + 3], targ_pad[T - 3: T + 3])
    win_sum -= tail.sum()

    n_win = T - 3
    similarity_loss = 1.0 - sim_sum / T
    progression_penalty = 1.0 - win_sum / (4.0 * n_win)
    loss = similarity_loss + 0.5 * progression_penalty
    return np.float32(loss)
